# revision 6
# baseline (speedup 1.0000x reference)
"""Trainium2 Bass kernel for LocalAttention (5-wide windowed attention).

Math (matches the reference exactly, but sparsely):
  center_b  = floor(sigmoid(dh_b . Wp + bp) * S); window = [left_b, right_b)
  scores outside the window are 0 (valid) or -1e9 (s >= len_b), so
  softmax/context reduce to:
    m   = max(window scores masked, 0 if any valid non-window else -inf)
    Z   = (len - nwinvalid) * exp(-m) + sum_j exp(s_j - m)
    ctx = [ exp(-m) * (vsum - sum_{winvalid} enc_j) + sum_j e_j enc_j ] / Z
  where vsum_b = sum_{s < len_b} enc[b, s, :]  -- the only memory-heavy term.

Device work per core (8-way batch-parallel, load balanced by length):
  - stream packed valid rows of enc (interleaved 4-rows-per-partition),
    quad-reduce on DVE, route rows->batch-slots via a small fp32 matmul
  - q = dh @ Wa on the PE; 5 window scores per batch via DVE dot
  - full softmax scalar chain + attention-weight row build on DVE/ACT
Host does only input packing/sharding (incl. the window position
computation that determines DMA addressing) and output gather/scatter.
"""

import sys

if "/opt/trn_rl_repo" not in sys.path:
    sys.path.insert(0, "/opt/trn_rl_repo")

import numpy as np

import concourse.bacc as bacc
import concourse.bass as bass
import concourse.tile as tile
from concourse import mybir
from concourse.bass_utils import run_bass_kernel_spmd

F32 = mybir.dt.float32
AF = mybir.ActivationFunctionType
OP = mybir.AluOpType

last_run = None  # BassKernelResults of the most recent kernel() call (for benching)
last_nc = None   # compiled Bacc module of the most recent kernel() call

B, S, H = 32, 4096, 512
WINDOW = 5
HALF = WINDOW // 2
NEG = -1.0e9
N_CORES = 8
GRP = 4          # rows interleaved per partition (quad-reduce on DVE)
TILE_ROWS = 512  # rows per streamed DMA tile ([128 partitions, 4, 512] f32 = 1MB)


def _centers(dh, Wp, bp):
    """Replicate the reference's f32 window-center computation bit-exactly
    (jax on CPU when available; numpy f32 fallback)."""
    try:
        import jax
        import jax.numpy as jnp

        cpu = jax.devices("cpu")[0]
        with jax.default_device(cpu):
            pos = jax.nn.sigmoid(jnp.asarray(dh) @ jnp.asarray(Wp).T + jnp.asarray(bp))[:, 0] * S
            return np.asarray(jnp.floor(pos).astype(jnp.int32))
    except Exception:
        z = (dh.astype(np.float32) @ Wp.T.astype(np.float32) + bp.astype(np.float32))[:, 0]
        pos = (np.float32(1.0) / (np.float32(1.0) + np.exp(-z.astype(np.float32)))) * np.float32(S)
        return np.floor(pos).astype(np.int32)


def _assign_batches(lens):
    """Greedy LPT assignment of batches to cores, balancing total rows."""
    order = np.argsort(-lens, kind="stable")
    loads = [0] * N_CORES
    slots = [[] for _ in range(N_CORES)]
    for b in order:
        c = int(np.argmin(loads))
        slots[c].append(int(b))
        loads[c] += int(lens[b])
    ns = max(len(s) for s in slots)
    if ns * WINDOW > 128:  # qrep matmul needs NS*5 output partitions <= 128
        slots = [list(range(4 * c, 4 * c + 4)) for c in range(N_CORES)]
        ns = 4
    return slots, ns


def _build_program(NS, TBIG):
    NS5 = NS * WINDOW
    nc = bacc.Bacc("TRN2", target_bir_lowering=False, debug=False, num_devices=N_CORES)

    # ---- DRAM I/O ----------------------------------------------------------
    penc = nc.dram_tensor("penc", [TBIG, 128, GRP, H], F32, kind="ExternalInput")
    selg = nc.dram_tensor("selg", [TBIG, 128, NS], F32, kind="ExternalInput")
    wenc = nc.dram_tensor("wenc", [NS5, H], F32, kind="ExternalInput")
    dhT = nc.dram_tensor("dhT", [H, NS], F32, kind="ExternalInput")
    wa = nc.dram_tensor("wa", [H, H], F32, kind="ExternalInput")
    bsel = nc.dram_tensor("bsel", [NS, NS5], F32, kind="ExternalInput")
    iota = nc.dram_tensor("iota", [NS, S], F32, kind="ExternalInput")
    ident = nc.dram_tensor("ident", [128, 128], F32, kind="ExternalInput")
    lenf = nc.dram_tensor("lenf", [NS, 1], F32, kind="ExternalInput")
    nonwin = nc.dram_tensor("nonwin", [1, NS], F32, kind="ExternalInput")
    mfloor = nc.dram_tensor("mfloor", [1, NS], F32, kind="ExternalInput")
    vwin = nc.dram_tensor("vwin", [1, NS5], F32, kind="ExternalInput")
    negoff = nc.dram_tensor("negoff", [1, NS5], F32, kind="ExternalInput")
    vwblk = nc.dram_tensor("vwblk", [NS5, NS], F32, kind="ExternalInput")
    eblkm = nc.dram_tensor("eblkm", [NS5, NS], F32, kind="ExternalInput")

    octx = nc.dram_tensor("octx", [NS, H], F32, kind="ExternalOutput")
    oattn = nc.dram_tensor("oattn", [NS, S], F32, kind="ExternalOutput")
    owwin = nc.dram_tensor("owwin", [1, NS5], F32, kind="ExternalOutput")

    with tile.TileContext(nc) as tc:
        with tc.tile_pool(name="consts", bufs=1) as cp, \
             tc.tile_pool(name="stream", bufs=6) as sp, \
             tc.tile_pool(name="acc", bufs=3) as ap_, \
             tc.tile_pool(name="work", bufs=1) as wp, \
             tc.tile_pool(name="psA", bufs=1, space="PSUM") as psA, \
             tc.tile_pool(name="psB", bufs=2, space="PSUM") as psB, \
             tc.tile_pool(name="psC", bufs=3, space="PSUM") as psC:

            # ---- constants -------------------------------------------------
            selg_sb = cp.tile([128, TBIG, NS], F32)
            nc.sync.dma_start(out=selg_sb[:], in_=selg[:].rearrange("t p s -> p t s"))
            wa_sb = cp.tile([128, 4, H], F32)
            nc.sync.dma_start(out=wa_sb[:], in_=wa[:].rearrange("(k p) h -> p k h", p=128))
            dht_sb = cp.tile([128, 4, NS], F32)
            nc.sync.dma_start(out=dht_sb[:], in_=dhT[:].rearrange("(k p) s -> p k s", p=128))
            wenc_sb = cp.tile([NS5, H], F32)
            nc.sync.dma_start(out=wenc_sb[:], in_=wenc[:])
            bsel_sb = cp.tile([NS, NS5], F32)
            nc.sync.dma_start(out=bsel_sb[:], in_=bsel[:])
            iota_sb = cp.tile([NS, S], F32)
            nc.sync.dma_start(out=iota_sb[:], in_=iota[:])
            ident_sb = cp.tile([128, 128], F32)
            nc.sync.dma_start(out=ident_sb[:], in_=ident[:])
            lenf_sb = cp.tile([NS, 1], F32)
            nc.sync.dma_start(out=lenf_sb[:], in_=lenf[:])
            nonwin_sb = cp.tile([1, NS], F32)
            nc.sync.dma_start(out=nonwin_sb[:], in_=nonwin[:])
            mfloor_sb = cp.tile([1, NS], F32)
            nc.sync.dma_start(out=mfloor_sb[:], in_=mfloor[:])
            vwin_sb = cp.tile([1, NS5], F32)
            nc.sync.dma_start(out=vwin_sb[:], in_=vwin[:])
            negoff_sb = cp.tile([1, NS5], F32)
            nc.sync.dma_start(out=negoff_sb[:], in_=negoff[:])
            vwblk_sb = cp.tile([NS5, NS], F32)
            nc.sync.dma_start(out=vwblk_sb[:], in_=vwblk[:])
            eblkm_sb = cp.tile([NS5, NS], F32)
            nc.sync.dma_start(out=eblkm_sb[:], in_=eblkm[:])

            # ---- big stream: vsum[slot] = sum of that slot's valid rows ----
            vsum_ps = psA.tile([NS, H], F32, tag="vsum")
            for g in range(TBIG):
                t = sp.tile([128, GRP, H], F32, tag="stream")
                nc.sync.dma_start(out=t[:], in_=penc[g])
                a1 = ap_.tile([128, H], F32, tag="a1")
                a2 = ap_.tile([128, H], F32, tag="a2")
                q4 = ap_.tile([128, H], F32, tag="q4")
                nc.vector.tensor_tensor(out=a1[:], in0=t[:, 0, :], in1=t[:, 1, :], op=OP.add)
                nc.vector.tensor_tensor(out=a2[:], in0=t[:, 2, :], in1=t[:, 3, :], op=OP.add)
                nc.vector.tensor_tensor(out=q4[:], in0=a1[:], in1=a2[:], op=OP.add)
                nc.tensor.matmul(vsum_ps[:], selg_sb[:, g, :], q4[:],
                                 start=(g == 0), stop=(g == TBIG - 1),
                                 skip_group_check=True)

            # ---- q = dh @ Wa  -> window scores ----------------------------
            q_ps = psB.tile([NS, H], F32, tag="bigtmp")
            for k in range(4):
                nc.tensor.matmul(q_ps[:], dht_sb[:, k, :], wa_sb[:, k, :],
                                 start=(k == 0), stop=(k == 3), skip_group_check=True)
            q_sb = wp.tile([NS, H], F32)
            nc.scalar.copy(out=q_sb[:], in_=q_ps[:])

            qrep_ps = psB.tile([NS5, H], F32, tag="bigtmp")
            nc.tensor.matmul(qrep_ps[:], bsel_sb[:], q_sb[:], start=True, stop=True,
                             skip_group_check=True)
            prod_sb = wp.tile([NS5, H], F32)
            nc.vector.tensor_tensor(out=prod_sb[:], in0=wenc_sb[:], in1=qrep_ps[:], op=OP.mult)
            svec_sb = wp.tile([NS5, 1], F32)
            nc.vector.reduce_sum(out=svec_sb[:], in_=prod_sb[:], axis=mybir.AxisListType.X)

            # scores to free axis: s_row[0, slot*5+j]
            srow_ps = psC.tile([1, NS5], F32, tag="smalltmp")
            nc.tensor.transpose(srow_ps[:], svec_sb[:], ident_sb[:NS5, :NS5])
            srow_sb = wp.tile([1, NS5], F32)
            nc.scalar.copy(out=srow_sb[:], in_=srow_ps[:])

            # masked scores: s*vwin + (vwin-1)*1e9
            sm_sb = wp.tile([1, NS5], F32)
            nc.vector.tensor_tensor(out=sm_sb[:], in0=srow_sb[:], in1=vwin_sb[:], op=OP.mult)
            nc.vector.tensor_tensor(out=sm_sb[:], in0=sm_sb[:], in1=negoff_sb[:], op=OP.add)

            # m = max(max_j s_masked, mfloor)
            mrow_sb = wp.tile([1, NS], F32)
            nc.vector.reduce_max(out=mrow_sb[:],
                                 in_=sm_sb[:].rearrange("p (s j) -> p s j", j=WINDOW),
                                 axis=mybir.AxisListType.X)
            nc.vector.tensor_tensor(out=mrow_sb[:], in0=mrow_sb[:], in1=mfloor_sb[:], op=OP.max)

            # broadcast m to 5 window lanes via PE: m5 = mcol.T-route through bsel
            mcol_ps = psC.tile([NS, 1], F32, tag="smalltmp")
            nc.tensor.transpose(mcol_ps[:], mrow_sb[:], ident_sb[:1, :1])
            mcol_sb = wp.tile([NS, 1], F32)
            nc.scalar.copy(out=mcol_sb[:], in_=mcol_ps[:])
            m5_ps = psC.tile([1, NS5], F32, tag="smalltmp")
            nc.tensor.matmul(m5_ps[:], mcol_sb[:], bsel_sb[:], start=True, stop=True,
                             skip_group_check=True)

            # e = exp(s_masked - m)
            d_sb = wp.tile([1, NS5], F32)
            nc.vector.tensor_tensor(out=d_sb[:], in0=sm_sb[:], in1=m5_ps[:], op=OP.subtract)
            e_sb = wp.tile([1, NS5], F32)
            nc.scalar.activation(out=e_sb[:], in_=d_sb[:], func=AF.Exp)

            # E = sum_j e ; expm = exp(-m) ; Z = nonwin*expm + E ; inv = 1/Z
            E_sb = wp.tile([1, NS], F32)
            nc.vector.reduce_sum(out=E_sb[:],
                                 in_=e_sb[:].rearrange("p (s j) -> p s j", j=WINDOW),
                                 axis=mybir.AxisListType.X)
            expm_sb = wp.tile([1, NS], F32)
            nc.scalar.activation(out=expm_sb[:], in_=mrow_sb[:], func=AF.Exp, scale=-1.0)
            z_sb = wp.tile([1, NS], F32)
            nc.vector.tensor_tensor(out=z_sb[:], in0=nonwin_sb[:], in1=expm_sb[:], op=OP.mult)
            nc.vector.tensor_tensor(out=z_sb[:], in0=z_sb[:], in1=E_sb[:], op=OP.add)
            inv_sb = wp.tile([1, NS], F32)
            nc.vector.reciprocal(out=inv_sb[:], in_=z_sb[:])
            t1_sb = wp.tile([1, NS], F32)  # base weight = exp(-m)/Z
            nc.vector.tensor_tensor(out=t1_sb[:], in0=inv_sb[:], in1=expm_sb[:], op=OP.mult)

            # per-partition scalar columns
            t1c_ps = psC.tile([NS, 1], F32, tag="smalltmp")
            nc.tensor.transpose(t1c_ps[:], t1_sb[:], ident_sb[:1, :1])
            t1c_sb = wp.tile([NS, 1], F32)
            nc.scalar.copy(out=t1c_sb[:], in_=t1c_ps[:])
            invc_ps = psC.tile([NS, 1], F32, tag="smalltmp")
            nc.tensor.transpose(invc_ps[:], inv_sb[:], ident_sb[:1, :1])
            invc_sb = wp.tile([NS, 1], F32)
            nc.scalar.copy(out=invc_sb[:], in_=invc_ps[:])

            # window weights wwin = e * inv (broadcast inv via PE)
            inv5_ps = psC.tile([1, NS5], F32, tag="smalltmp")
            nc.tensor.matmul(inv5_ps[:], invc_sb[:], bsel_sb[:], start=True, stop=True,
                             skip_group_check=True)
            wwin_sb = wp.tile([1, NS5], F32)
            nc.vector.tensor_tensor(out=wwin_sb[:], in0=e_sb[:], in1=inv5_ps[:], op=OP.mult)
            nc.sync.dma_start(out=owwin[:], in_=wwin_sb[:])

            # eblk[k, slot] = e_k * (k//5 == slot) for the C matmul
            eT_ps = psC.tile([NS5, 1], F32, tag="smalltmp")
            nc.tensor.transpose(eT_ps[:], e_sb[:], ident_sb[:1, :1])
            eT_sb = wp.tile([NS5, 1], F32)
            nc.scalar.copy(out=eT_sb[:], in_=eT_ps[:])
            eblk_sb = wp.tile([NS5, NS], F32)
            nc.vector.tensor_scalar(out=eblk_sb[:], in0=eblkm_sb[:],
                                    scalar1=eT_sb[:], scalar2=None, op0=OP.mult)

            # A = sum_{winvalid} enc_j ; C = sum e_j * enc_j
            a_ps = psB.tile([NS, H], F32, tag="AC")
            nc.tensor.matmul(a_ps[:], vwblk_sb[:], wenc_sb[:], start=True, stop=True,
                             skip_group_check=True)
            c_ps = psB.tile([NS, H], F32, tag="AC")
            nc.tensor.matmul(c_ps[:], eblk_sb[:], wenc_sb[:], start=True, stop=True,
                             skip_group_check=True)

            # ctx = (vsum - A) * t1 + C * inv
            vsum_sb = wp.tile([NS, H], F32)
            nc.scalar.copy(out=vsum_sb[:], in_=vsum_ps[:])
            d1_sb = wp.tile([NS, H], F32)
            nc.vector.tensor_tensor(out=d1_sb[:], in0=vsum_sb[:], in1=a_ps[:], op=OP.subtract)
            nc.vector.tensor_scalar(out=d1_sb[:], in0=d1_sb[:],
                                    scalar1=t1c_sb[:], scalar2=None, op0=OP.mult)
            d3_sb = wp.tile([NS, H], F32)
            nc.vector.tensor_scalar(out=d3_sb[:], in0=c_ps[:],
                                    scalar1=invc_sb[:], scalar2=None, op0=OP.mult)
            ctx_sb = wp.tile([NS, H], F32)
            nc.vector.tensor_tensor(out=ctx_sb[:], in0=d1_sb[:], in1=d3_sb[:], op=OP.add)
            nc.sync.dma_start(out=octx[:], in_=ctx_sb[:])

            # attn base row: (iota < len) * base ; window region fixed on host
            attn_sb = wp.tile([NS, S], F32)
            nc.vector.tensor_scalar(out=attn_sb[:], in0=iota_sb[:],
                                    scalar1=lenf_sb[:], scalar2=t1c_sb[:],
                                    op0=OP.is_lt, op1=OP.mult)
            nc.sync.dma_start(out=oattn[:], in_=attn_sb[:])

    nc.compile()
    return nc


def kernel(**inputs):
    dh = np.ascontiguousarray(np.asarray(inputs["decoder_hidden"], dtype=np.float32))
    enc = np.ascontiguousarray(np.asarray(inputs["encoder_outputs"], dtype=np.float32))
    Wp = np.asarray(inputs["Wp"], dtype=np.float32)
    bp = np.asarray(inputs["bp"], dtype=np.float32)
    Wa = np.ascontiguousarray(np.asarray(inputs["Wa"], dtype=np.float32))
    lens = np.asarray(inputs["input_lengths"]).astype(np.int64)

    centers = _centers(dh, Wp, bp).astype(np.int64)
    left = np.maximum(0, centers - HALF)
    right = np.minimum(S, centers + HALF + 1)
    width = right - left

    slots, NS = _assign_batches(lens)
    NS5 = NS * WINDOW

    # rows per core with each slot padded to a multiple of GRP rows
    rows_c = [sum(int(-(-lens[b] // GRP) * GRP) for b in sl) for sl in slots]
    TBIG = max(1, -(-max(rows_c) // TILE_ROWS))
    R = TBIG * TILE_ROWS

    # ---- shared constants --------------------------------------------------
    bsel_h = np.zeros((NS, NS5), np.float32)
    for s_ in range(NS):
        bsel_h[s_, s_ * WINDOW:(s_ + 1) * WINDOW] = 1.0
    iota_h = np.broadcast_to(np.arange(S, dtype=np.float32), (NS, S)).copy()
    ident_h = np.eye(128, dtype=np.float32)
    eblkm_h = np.zeros((NS5, NS), np.float32)
    for s_ in range(NS):
        eblkm_h[s_ * WINDOW:(s_ + 1) * WINDOW, s_] = 1.0

    in_maps = []
    for c in range(N_CORES):
        sl = slots[c]
        packed = np.zeros((R, H), np.float32)
        rowslot = np.full(R, -1, np.int64)
        wenc_h = np.zeros((NS5, H), np.float32)
        dhT_h = np.zeros((H, NS), np.float32)
        lenf_h = np.zeros((NS, 1), np.float32)
        nonwin_h = np.ones((1, NS), np.float32)
        mfloor_h = np.zeros((1, NS), np.float32)
        vwin_h = np.zeros((1, NS5), np.float32)
        vwblk_h = np.zeros((NS5, NS), np.float32)
        off = 0
        for s_, b in enumerate(sl):
            L = int(lens[b])
            packed[off:off + L] = enc[b, :L]
            rowslot[off:off + L] = s_
            off += -(-L // GRP) * GRP
            dhT_h[:, s_] = dh[b]
            lenf_h[s_, 0] = np.float32(L)
            nwv = 0
            for j in range(WINDOW):
                p = int(left[b]) + j
                if j < int(width[b]) and p < L:
                    wenc_h[s_ * WINDOW + j] = enc[b, p]
                    vwin_h[0, s_ * WINDOW + j] = 1.0
                    vwblk_h[s_ * WINDOW + j, s_] = 1.0
                    nwv += 1
            nonwin_h[0, s_] = np.float32(L - nwv)
            mfloor_h[0, s_] = np.float32(0.0) if (L - nwv) > 0 else np.float32(-1e30)
        negoff_h = (vwin_h - 1.0) * np.float32(1e9)

        # interleaved layout: row (g*512 + p*4 + k) -> penc[g, p, k, :]
        penc_h = packed.reshape(TBIG, 128, GRP, H)
        gs = rowslot.reshape(TBIG, 128, GRP)[:, :, 0]  # slot of each 4-row group
        selg_h = np.zeros((TBIG, 128, NS), np.float32)
        valid = gs >= 0
        gi, pi = np.nonzero(valid)
        selg_h[gi, pi, gs[gi, pi]] = 1.0

        in_maps.append({
            "penc": penc_h, "selg": selg_h, "wenc": wenc_h, "dhT": dhT_h,
            "wa": Wa, "bsel": bsel_h, "iota": iota_h, "ident": ident_h,
            "lenf": lenf_h, "nonwin": nonwin_h, "mfloor": mfloor_h,
            "vwin": vwin_h, "negoff": negoff_h, "vwblk": vwblk_h,
            "eblkm": eblkm_h,
        })

    nc = _build_program(NS, TBIG)
    global last_nc, last_run
    last_nc = nc
    _r = run_bass_kernel_spmd(nc, in_maps, list(range(N_CORES)))
    last_run = _r
    res = _r.results

    ctx_full = np.zeros((B, H), np.float32)
    attn_full = np.zeros((B, S), np.float32)
    for c in range(N_CORES):
        for s_, b in enumerate(slots[c]):
            ctx_full[b] = res[c]["octx"][s_]
            attn_full[b] = res[c]["oattn"][s_]
            w = int(width[b])
            lo = int(left[b])
            attn_full[b, lo:lo + w] = res[c]["owwin"][0, s_ * WINDOW:s_ * WINDOW + w]
    return ctx_full, attn_full


# revision 8
# speedup vs baseline: 1.0147x; 1.0147x over previous
"""Trainium2 Bass kernel for LocalAttention (5-wide windowed attention).

Math (matches the reference exactly, but sparsely):
  center_b  = floor(sigmoid(dh_b . Wp + bp) * S); window = [left_b, right_b)
  scores outside the window are 0 (valid) or -1e9 (s >= len_b), so
  softmax/context reduce to:
    m   = max(window scores masked, 0 if any valid non-window else -inf)
    Z   = (len - nwinvalid) * exp(-m) + sum_j exp(s_j - m)
    ctx = [ exp(-m) * (vsum - sum_{winvalid} enc_j) + sum_j e_j enc_j ] / Z
  where vsum_b = sum_{s < len_b} enc[b, s, :]  -- the only memory-heavy term.

Device work per core (8-way batch-parallel, load balanced by length):
  - stream packed valid rows of enc (interleaved 4-rows-per-partition),
    quad-reduce on DVE, route rows->batch-slots via a small fp32 matmul
  - q = dh @ Wa on the PE; 5 window scores per batch via DVE dot
  - full softmax scalar chain + attention-weight row build on DVE/ACT
Host does only input packing/sharding (incl. the window position
computation that determines DMA addressing) and output gather/scatter.
"""

import sys

if "/opt/trn_rl_repo" not in sys.path:
    sys.path.insert(0, "/opt/trn_rl_repo")

import numpy as np

import concourse.bacc as bacc
import concourse.bass as bass
import concourse.tile as tile
from concourse import mybir
from concourse.bass_utils import run_bass_kernel_spmd

F32 = mybir.dt.float32
AF = mybir.ActivationFunctionType
OP = mybir.AluOpType

last_run = None  # BassKernelResults of the most recent kernel() call (for benching)
last_nc = None   # compiled Bacc module of the most recent kernel() call

B, S, H = 32, 4096, 512
WINDOW = 5
HALF = WINDOW // 2
NEG = -1.0e9
N_CORES = 8
GRP = 4          # rows interleaved per partition (quad-reduce on DVE)
TILE_ROWS = 512  # rows per streamed DMA tile ([128 partitions, 4, 512] f32 = 1MB)


def _centers(dh, Wp, bp):
    """Replicate the reference's f32 window-center computation bit-exactly
    (jax on CPU when available; numpy f32 fallback)."""
    try:
        import jax
        import jax.numpy as jnp

        cpu = jax.devices("cpu")[0]
        with jax.default_device(cpu):
            pos = jax.nn.sigmoid(jnp.asarray(dh) @ jnp.asarray(Wp).T + jnp.asarray(bp))[:, 0] * S
            return np.asarray(jnp.floor(pos).astype(jnp.int32))
    except Exception:
        z = (dh.astype(np.float32) @ Wp.T.astype(np.float32) + bp.astype(np.float32))[:, 0]
        pos = (np.float32(1.0) / (np.float32(1.0) + np.exp(-z.astype(np.float32)))) * np.float32(S)
        return np.floor(pos).astype(np.int32)


def _assign_batches(lens):
    """Greedy LPT assignment of batches to cores, balancing total rows."""
    order = np.argsort(-lens, kind="stable")
    loads = [0] * N_CORES
    slots = [[] for _ in range(N_CORES)]
    for b in order:
        c = int(np.argmin(loads))
        slots[c].append(int(b))
        loads[c] += int(lens[b])
    ns = max(len(s) for s in slots)
    if ns * WINDOW > 128:  # qrep matmul needs NS*5 output partitions <= 128
        slots = [list(range(4 * c, 4 * c + 4)) for c in range(N_CORES)]
        ns = 4
    return slots, ns


def _build_program(NS, TBIG):
    NS5 = NS * WINDOW
    nc = bacc.Bacc("TRN2", target_bir_lowering=False, debug=False, num_devices=N_CORES)

    # ---- DRAM I/O ----------------------------------------------------------
    penc = nc.dram_tensor("penc", [TBIG, 128, GRP, H], F32, kind="ExternalInput")
    selg = nc.dram_tensor("selg", [TBIG, 128, NS], F32, kind="ExternalInput")
    wenc = nc.dram_tensor("wenc", [NS5, H], F32, kind="ExternalInput")
    dhT = nc.dram_tensor("dhT", [H, NS], F32, kind="ExternalInput")
    wa = nc.dram_tensor("wa", [H, H], F32, kind="ExternalInput")
    bsel = nc.dram_tensor("bsel", [NS, NS5], F32, kind="ExternalInput")
    iota = nc.dram_tensor("iota", [NS, S], F32, kind="ExternalInput")
    ident = nc.dram_tensor("ident", [128, 128], F32, kind="ExternalInput")
    lenf = nc.dram_tensor("lenf", [NS, 1], F32, kind="ExternalInput")
    nonwin = nc.dram_tensor("nonwin", [1, NS], F32, kind="ExternalInput")
    mfloor = nc.dram_tensor("mfloor", [1, NS], F32, kind="ExternalInput")
    vwin = nc.dram_tensor("vwin", [1, NS5], F32, kind="ExternalInput")
    negoff = nc.dram_tensor("negoff", [1, NS5], F32, kind="ExternalInput")
    vwblk = nc.dram_tensor("vwblk", [NS5, NS], F32, kind="ExternalInput")
    eblkm = nc.dram_tensor("eblkm", [NS5, NS], F32, kind="ExternalInput")

    octx = nc.dram_tensor("octx", [NS, H], F32, kind="ExternalOutput")
    oattn = nc.dram_tensor("oattn", [NS, S], F32, kind="ExternalOutput")
    owwin = nc.dram_tensor("owwin", [1, NS5], F32, kind="ExternalOutput")

    with tile.TileContext(nc) as tc:
        with tc.tile_pool(name="consts", bufs=1) as cp, \
             tc.tile_pool(name="stream", bufs=6) as sp, \
             tc.tile_pool(name="acc", bufs=3) as ap_, \
             tc.tile_pool(name="work", bufs=1) as wp, \
             tc.tile_pool(name="psA", bufs=1, space="PSUM") as psA, \
             tc.tile_pool(name="psB", bufs=2, space="PSUM") as psB, \
             tc.tile_pool(name="psC", bufs=3, space="PSUM") as psC:

            # ---- constants -------------------------------------------------
            selg_sb = cp.tile([128, TBIG, NS], F32)
            nc.sync.dma_start(out=selg_sb[:], in_=selg[:].rearrange("t p s -> p t s"))
            wa_sb = cp.tile([128, 4, H], F32)
            nc.sync.dma_start(out=wa_sb[:], in_=wa[:].rearrange("(k p) h -> p k h", p=128))
            dht_sb = cp.tile([128, 4, NS], F32)
            nc.sync.dma_start(out=dht_sb[:], in_=dhT[:].rearrange("(k p) s -> p k s", p=128))
            wenc_sb = cp.tile([NS5, H], F32)
            nc.sync.dma_start(out=wenc_sb[:], in_=wenc[:])
            bsel_sb = cp.tile([NS, NS5], F32)
            nc.sync.dma_start(out=bsel_sb[:], in_=bsel[:])
            iota_sb = cp.tile([NS, S], F32)
            nc.sync.dma_start(out=iota_sb[:], in_=iota[:])
            ident_sb = cp.tile([128, 128], F32)
            nc.sync.dma_start(out=ident_sb[:], in_=ident[:])
            lenf_sb = cp.tile([NS, 1], F32)
            nc.sync.dma_start(out=lenf_sb[:], in_=lenf[:])
            nonwin_sb = cp.tile([1, NS], F32)
            nc.sync.dma_start(out=nonwin_sb[:], in_=nonwin[:])
            mfloor_sb = cp.tile([1, NS], F32)
            nc.sync.dma_start(out=mfloor_sb[:], in_=mfloor[:])
            vwin_sb = cp.tile([1, NS5], F32)
            nc.sync.dma_start(out=vwin_sb[:], in_=vwin[:])
            negoff_sb = cp.tile([1, NS5], F32)
            nc.sync.dma_start(out=negoff_sb[:], in_=negoff[:])
            vwblk_sb = cp.tile([NS5, NS], F32)
            nc.sync.dma_start(out=vwblk_sb[:], in_=vwblk[:])
            eblkm_sb = cp.tile([NS5, NS], F32)
            nc.sync.dma_start(out=eblkm_sb[:], in_=eblkm[:])

            # ---- big stream: vsum[slot] = sum of that slot's valid rows ----
            vsum_ps = psA.tile([NS, H], F32, tag="vsum")
            for g in range(TBIG):
                t = sp.tile([128, GRP, H], F32, tag="stream")
                nc.sync.dma_start(out=t[:], in_=penc[g])
                a1 = ap_.tile([128, H], F32, tag="a1")
                a2 = ap_.tile([128, H], F32, tag="a2")
                q4 = ap_.tile([128, H], F32, tag="q4")
                # high priority: free stream-tile slots promptly so DMA never
                # stalls on pool bufs (DVE otherwise runs the scalar chain first)
                with tc.high_priority(offset=1_000_000):
                    nc.vector.tensor_tensor(out=a1[:], in0=t[:, 0, :], in1=t[:, 1, :], op=OP.add)
                    nc.vector.tensor_tensor(out=a2[:], in0=t[:, 2, :], in1=t[:, 3, :], op=OP.add)
                    nc.vector.tensor_tensor(out=q4[:], in0=a1[:], in1=a2[:], op=OP.add)
                nc.tensor.matmul(vsum_ps[:], selg_sb[:, g, :], q4[:],
                                 start=(g == 0), stop=(g == TBIG - 1),
                                 skip_group_check=True)

            # ---- q = dh @ Wa  -> window scores ----------------------------
            q_ps = psB.tile([NS, H], F32, tag="bigtmp")
            for k in range(4):
                nc.tensor.matmul(q_ps[:], dht_sb[:, k, :], wa_sb[:, k, :],
                                 start=(k == 0), stop=(k == 3), skip_group_check=True)
            q_sb = wp.tile([NS, H], F32)
            nc.scalar.copy(out=q_sb[:], in_=q_ps[:])

            qrep_ps = psB.tile([NS5, H], F32, tag="bigtmp")
            nc.tensor.matmul(qrep_ps[:], bsel_sb[:], q_sb[:], start=True, stop=True,
                             skip_group_check=True)
            prod_sb = wp.tile([NS5, H], F32)
            nc.vector.tensor_tensor(out=prod_sb[:], in0=wenc_sb[:], in1=qrep_ps[:], op=OP.mult)
            svec_sb = wp.tile([NS5, 1], F32)
            nc.vector.reduce_sum(out=svec_sb[:], in_=prod_sb[:], axis=mybir.AxisListType.X)

            # scores to free axis: s_row[0, slot*5+j]
            srow_ps = psC.tile([1, NS5], F32, tag="smalltmp")
            nc.tensor.transpose(srow_ps[:], svec_sb[:], ident_sb[:NS5, :NS5])
            srow_sb = wp.tile([1, NS5], F32)
            nc.scalar.copy(out=srow_sb[:], in_=srow_ps[:])

            # masked scores: s*vwin + (vwin-1)*1e9
            sm_sb = wp.tile([1, NS5], F32)
            nc.vector.tensor_tensor(out=sm_sb[:], in0=srow_sb[:], in1=vwin_sb[:], op=OP.mult)
            nc.vector.tensor_tensor(out=sm_sb[:], in0=sm_sb[:], in1=negoff_sb[:], op=OP.add)

            # m = max(max_j s_masked, mfloor)
            mrow_sb = wp.tile([1, NS], F32)
            nc.vector.reduce_max(out=mrow_sb[:],
                                 in_=sm_sb[:].rearrange("p (s j) -> p s j", j=WINDOW),
                                 axis=mybir.AxisListType.X)
            nc.vector.tensor_tensor(out=mrow_sb[:], in0=mrow_sb[:], in1=mfloor_sb[:], op=OP.max)

            # broadcast m to 5 window lanes via PE: m5 = mcol.T-route through bsel
            mcol_ps = psC.tile([NS, 1], F32, tag="smalltmp")
            nc.tensor.transpose(mcol_ps[:], mrow_sb[:], ident_sb[:1, :1])
            mcol_sb = wp.tile([NS, 1], F32)
            nc.scalar.copy(out=mcol_sb[:], in_=mcol_ps[:])
            m5_ps = psC.tile([1, NS5], F32, tag="smalltmp")
            nc.tensor.matmul(m5_ps[:], mcol_sb[:], bsel_sb[:], start=True, stop=True,
                             skip_group_check=True)

            # e = exp(s_masked - m)
            d_sb = wp.tile([1, NS5], F32)
            nc.vector.tensor_tensor(out=d_sb[:], in0=sm_sb[:], in1=m5_ps[:], op=OP.subtract)
            e_sb = wp.tile([1, NS5], F32)
            nc.scalar.activation(out=e_sb[:], in_=d_sb[:], func=AF.Exp)

            # E = sum_j e ; expm = exp(-m) ; Z = nonwin*expm + E ; inv = 1/Z
            E_sb = wp.tile([1, NS], F32)
            nc.vector.reduce_sum(out=E_sb[:],
                                 in_=e_sb[:].rearrange("p (s j) -> p s j", j=WINDOW),
                                 axis=mybir.AxisListType.X)
            expm_sb = wp.tile([1, NS], F32)
            nc.scalar.activation(out=expm_sb[:], in_=mrow_sb[:], func=AF.Exp, scale=-1.0)
            z_sb = wp.tile([1, NS], F32)
            nc.vector.tensor_tensor(out=z_sb[:], in0=nonwin_sb[:], in1=expm_sb[:], op=OP.mult)
            nc.vector.tensor_tensor(out=z_sb[:], in0=z_sb[:], in1=E_sb[:], op=OP.add)
            inv_sb = wp.tile([1, NS], F32)
            nc.vector.reciprocal(out=inv_sb[:], in_=z_sb[:])
            t1_sb = wp.tile([1, NS], F32)  # base weight = exp(-m)/Z
            nc.vector.tensor_tensor(out=t1_sb[:], in0=inv_sb[:], in1=expm_sb[:], op=OP.mult)

            # per-partition scalar columns
            t1c_ps = psC.tile([NS, 1], F32, tag="smalltmp")
            nc.tensor.transpose(t1c_ps[:], t1_sb[:], ident_sb[:1, :1])
            t1c_sb = wp.tile([NS, 1], F32)
            nc.scalar.copy(out=t1c_sb[:], in_=t1c_ps[:])
            invc_ps = psC.tile([NS, 1], F32, tag="smalltmp")
            nc.tensor.transpose(invc_ps[:], inv_sb[:], ident_sb[:1, :1])
            invc_sb = wp.tile([NS, 1], F32)
            nc.scalar.copy(out=invc_sb[:], in_=invc_ps[:])

            # window weights wwin = e * inv (broadcast inv via PE)
            inv5_ps = psC.tile([1, NS5], F32, tag="smalltmp")
            nc.tensor.matmul(inv5_ps[:], invc_sb[:], bsel_sb[:], start=True, stop=True,
                             skip_group_check=True)
            wwin_sb = wp.tile([1, NS5], F32)
            nc.vector.tensor_tensor(out=wwin_sb[:], in0=e_sb[:], in1=inv5_ps[:], op=OP.mult)
            nc.sync.dma_start(out=owwin[:], in_=wwin_sb[:])

            # eblk[k, slot] = e_k * (k//5 == slot) for the C matmul
            eT_ps = psC.tile([NS5, 1], F32, tag="smalltmp")
            nc.tensor.transpose(eT_ps[:], e_sb[:], ident_sb[:1, :1])
            eT_sb = wp.tile([NS5, 1], F32)
            nc.scalar.copy(out=eT_sb[:], in_=eT_ps[:])
            eblk_sb = wp.tile([NS5, NS], F32)
            nc.vector.tensor_scalar(out=eblk_sb[:], in0=eblkm_sb[:],
                                    scalar1=eT_sb[:], scalar2=None, op0=OP.mult)

            # A = sum_{winvalid} enc_j ; C = sum e_j * enc_j
            a_ps = psB.tile([NS, H], F32, tag="AC")
            nc.tensor.matmul(a_ps[:], vwblk_sb[:], wenc_sb[:], start=True, stop=True,
                             skip_group_check=True)
            c_ps = psB.tile([NS, H], F32, tag="AC")
            nc.tensor.matmul(c_ps[:], eblk_sb[:], wenc_sb[:], start=True, stop=True,
                             skip_group_check=True)

            # ctx = (vsum - A)*t1 + C*inv = vsum*t1 + (C*inv - A*t1)
            # h1 = C*inv - A*t1 is ready long before vsum, so the tail after the
            # final stream matmul is a single fused scalar_tensor_tensor.
            d3_sb = wp.tile([NS, H], F32)
            nc.vector.tensor_scalar(out=d3_sb[:], in0=c_ps[:],
                                    scalar1=invc_sb[:], scalar2=None, op0=OP.mult)
            at_sb = wp.tile([NS, H], F32)
            nc.vector.tensor_scalar(out=at_sb[:], in0=a_ps[:],
                                    scalar1=t1c_sb[:], scalar2=None, op0=OP.mult)
            h1_sb = wp.tile([NS, H], F32)
            nc.vector.tensor_tensor(out=h1_sb[:], in0=d3_sb[:], in1=at_sb[:], op=OP.subtract)
            ctx_sb = wp.tile([NS, H], F32)
            nc.vector.scalar_tensor_tensor(out=ctx_sb[:], in0=vsum_ps[:],
                                           scalar=t1c_sb[:], in1=h1_sb[:],
                                           op0=OP.mult, op1=OP.add)
            nc.sync.dma_start(out=octx[:], in_=ctx_sb[:])

            # attn base row: (iota < len) * base ; window region fixed on host
            attn_sb = wp.tile([NS, S], F32)
            nc.vector.tensor_scalar(out=attn_sb[:], in0=iota_sb[:],
                                    scalar1=lenf_sb[:], scalar2=t1c_sb[:],
                                    op0=OP.is_lt, op1=OP.mult)
            nc.sync.dma_start(out=oattn[:], in_=attn_sb[:])

    nc.compile()
    return nc


def kernel(**inputs):
    dh = np.ascontiguousarray(np.asarray(inputs["decoder_hidden"], dtype=np.float32))
    enc = np.ascontiguousarray(np.asarray(inputs["encoder_outputs"], dtype=np.float32))
    Wp = np.asarray(inputs["Wp"], dtype=np.float32)
    bp = np.asarray(inputs["bp"], dtype=np.float32)
    Wa = np.ascontiguousarray(np.asarray(inputs["Wa"], dtype=np.float32))
    lens = np.asarray(inputs["input_lengths"]).astype(np.int64)

    centers = _centers(dh, Wp, bp).astype(np.int64)
    left = np.maximum(0, centers - HALF)
    right = np.minimum(S, centers + HALF + 1)
    width = right - left

    slots, NS = _assign_batches(lens)
    NS5 = NS * WINDOW

    # rows per core with each slot padded to a multiple of GRP rows
    rows_c = [sum(int(-(-lens[b] // GRP) * GRP) for b in sl) for sl in slots]
    TBIG = max(1, -(-max(rows_c) // TILE_ROWS))
    R = TBIG * TILE_ROWS

    # ---- shared constants --------------------------------------------------
    bsel_h = np.zeros((NS, NS5), np.float32)
    for s_ in range(NS):
        bsel_h[s_, s_ * WINDOW:(s_ + 1) * WINDOW] = 1.0
    iota_h = np.broadcast_to(np.arange(S, dtype=np.float32), (NS, S)).copy()
    ident_h = np.eye(128, dtype=np.float32)
    eblkm_h = np.zeros((NS5, NS), np.float32)
    for s_ in range(NS):
        eblkm_h[s_ * WINDOW:(s_ + 1) * WINDOW, s_] = 1.0

    in_maps = []
    for c in range(N_CORES):
        sl = slots[c]
        packed = np.zeros((R, H), np.float32)
        rowslot = np.full(R, -1, np.int64)
        wenc_h = np.zeros((NS5, H), np.float32)
        dhT_h = np.zeros((H, NS), np.float32)
        lenf_h = np.zeros((NS, 1), np.float32)
        nonwin_h = np.ones((1, NS), np.float32)
        mfloor_h = np.zeros((1, NS), np.float32)
        vwin_h = np.zeros((1, NS5), np.float32)
        vwblk_h = np.zeros((NS5, NS), np.float32)
        off = 0
        for s_, b in enumerate(sl):
            L = int(lens[b])
            packed[off:off + L] = enc[b, :L]
            rowslot[off:off + L] = s_
            off += -(-L // GRP) * GRP
            dhT_h[:, s_] = dh[b]
            lenf_h[s_, 0] = np.float32(L)
            nwv = 0
            for j in range(WINDOW):
                p = int(left[b]) + j
                if j < int(width[b]) and p < L:
                    wenc_h[s_ * WINDOW + j] = enc[b, p]
                    vwin_h[0, s_ * WINDOW + j] = 1.0
                    vwblk_h[s_ * WINDOW + j, s_] = 1.0
                    nwv += 1
            nonwin_h[0, s_] = np.float32(L - nwv)
            mfloor_h[0, s_] = np.float32(0.0) if (L - nwv) > 0 else np.float32(-1e30)
        negoff_h = (vwin_h - 1.0) * np.float32(1e9)

        # interleaved layout: row (g*512 + p*4 + k) -> penc[g, p, k, :]
        penc_h = packed.reshape(TBIG, 128, GRP, H)
        gs = rowslot.reshape(TBIG, 128, GRP)[:, :, 0]  # slot of each 4-row group
        selg_h = np.zeros((TBIG, 128, NS), np.float32)
        valid = gs >= 0
        gi, pi = np.nonzero(valid)
        selg_h[gi, pi, gs[gi, pi]] = 1.0

        in_maps.append({
            "penc": penc_h, "selg": selg_h, "wenc": wenc_h, "dhT": dhT_h,
            "wa": Wa, "bsel": bsel_h, "iota": iota_h, "ident": ident_h,
            "lenf": lenf_h, "nonwin": nonwin_h, "mfloor": mfloor_h,
            "vwin": vwin_h, "negoff": negoff_h, "vwblk": vwblk_h,
            "eblkm": eblkm_h,
        })

    nc = _build_program(NS, TBIG)
    global last_nc, last_run
    last_nc = nc
    _r = run_bass_kernel_spmd(nc, in_maps, list(range(N_CORES)))
    last_run = _r
    res = _r.results

    ctx_full = np.zeros((B, H), np.float32)
    attn_full = np.zeros((B, S), np.float32)
    for c in range(N_CORES):
        for s_, b in enumerate(slots[c]):
            ctx_full[b] = res[c]["octx"][s_]
            attn_full[b] = res[c]["oattn"][s_]
            w = int(width[b])
            lo = int(left[b])
            attn_full[b, lo:lo + w] = res[c]["owwin"][0, s_ * WINDOW:s_ * WINDOW + w]
    return ctx_full, attn_full


# revision 9
# speedup vs baseline: 1.1220x; 1.1057x over previous
"""Trainium2 Bass kernel for LocalAttention (5-wide windowed attention).

Math (matches the reference exactly, but sparsely):
  center_b  = floor(sigmoid(dh_b . Wp + bp) * S); window = [left_b, right_b)
  scores outside the window are 0 (valid) or -1e9 (s >= len_b), so
  softmax/context reduce to:
    m   = max(window scores masked, 0 if any valid non-window else -inf)
    Z   = (len - nwinvalid) * exp(-m) + sum_j exp(s_j - m)
    ctx = [ exp(-m) * (vsum - sum_{winvalid} enc_j) + sum_j e_j enc_j ] / Z
  where vsum_b = sum_{s < len_b} enc[b, s, :]  -- the only memory-heavy term.

Device work per core (8-way batch-parallel, load balanced by length):
  - stream packed valid rows of enc (2 rows interleaved per partition),
    pair-reduce on DVE, route rows->batch-slots via a small fp32 matmul
  - q = dh @ Wa on the PE; 5 window scores per batch via DVE dot
  - full softmax scalar chain + attention-weight row build on DVE/ACT
Host does only input packing/sharding (incl. the window position
computation that determines DMA addressing) and output gather/scatter.
"""

import sys

if "/opt/trn_rl_repo" not in sys.path:
    sys.path.insert(0, "/opt/trn_rl_repo")

import numpy as np

import concourse.bacc as bacc
import concourse.bass as bass
import concourse.tile as tile
from concourse import mybir
from concourse.bass_utils import run_bass_kernel_spmd

F32 = mybir.dt.float32
AF = mybir.ActivationFunctionType
OP = mybir.AluOpType

last_run = None  # BassKernelResults of the most recent kernel() call (for benching)
last_nc = None   # compiled Bacc module of the most recent kernel() call

B, S, H = 32, 4096, 512
WINDOW = 5
HALF = WINDOW // 2
NEG = -1.0e9
N_CORES = 8
GRP = 2          # rows interleaved per partition (pair-reduce on DVE)
TILE_ROWS = 128 * GRP  # rows per streamed DMA tile ([128, GRP, 512] f32 = 512KB)


def _centers(dh, Wp, bp):
    """Replicate the reference's f32 window-center computation bit-exactly
    (jax on CPU when available; numpy f32 fallback)."""
    try:
        import jax
        import jax.numpy as jnp

        cpu = jax.devices("cpu")[0]
        with jax.default_device(cpu):
            pos = jax.nn.sigmoid(jnp.asarray(dh) @ jnp.asarray(Wp).T + jnp.asarray(bp))[:, 0] * S
            return np.asarray(jnp.floor(pos).astype(jnp.int32))
    except Exception:
        z = (dh.astype(np.float32) @ Wp.T.astype(np.float32) + bp.astype(np.float32))[:, 0]
        pos = (np.float32(1.0) / (np.float32(1.0) + np.exp(-z.astype(np.float32)))) * np.float32(S)
        return np.floor(pos).astype(np.int32)


def _assign_batches(lens):
    """Greedy LPT assignment of batches to cores, balancing total rows."""
    order = np.argsort(-lens, kind="stable")
    loads = [0] * N_CORES
    slots = [[] for _ in range(N_CORES)]
    for b in order:
        c = int(np.argmin(loads))
        slots[c].append(int(b))
        loads[c] += int(lens[b])
    ns = max(len(s) for s in slots)
    if ns * WINDOW > 128:  # qrep matmul needs NS*5 output partitions <= 128
        slots = [list(range(4 * c, 4 * c + 4)) for c in range(N_CORES)]
        ns = 4
    return slots, ns


def _build_program(NS, TB):
    """One SPMD program for all 8 cores. TB = number of streamed tiles.

    pack1 [128, F1]: Wa | ident | selg (TB*NS) | dhT (4*NS)
    pack2 [NS5, F2]: wenc | bsel | vwblk | eblkm | lenf | nonwin | mfloor
                     | vwin | negoff   (row-consts; see offsets below)
    """
    NS5 = NS * WINDOW
    F1 = 2048 + 128 + TB * NS + 4 * NS
    O_ID, O_SEL, O_DHT = 2048, 2048 + 128, 2048 + 128 + TB * NS
    F2 = 512 + NS5 + NS + NS + 1 + NS + NS + NS5 + NS5
    O_BSEL = 512
    O_VWB = O_BSEL + NS5
    O_EBM = O_VWB + NS
    O_LEN = O_EBM + NS
    O_NW = O_LEN + 1
    O_MF = O_NW + NS
    O_VW = O_MF + NS
    O_NO = O_VW + NS5

    nc = bacc.Bacc("TRN2", target_bir_lowering=False, debug=False, num_devices=N_CORES)

    penc = nc.dram_tensor("penc", [TB, 128, GRP, H], F32, kind="ExternalInput")
    pk1 = nc.dram_tensor("pk1", [128, F1], F32, kind="ExternalInput")
    pk2 = nc.dram_tensor("pk2", [NS5, F2], F32, kind="ExternalInput")

    octx = nc.dram_tensor("octx", [NS, H], F32, kind="ExternalOutput")
    oattn = nc.dram_tensor("oattn", [NS, S], F32, kind="ExternalOutput")
    owwin = nc.dram_tensor("owwin", [1, NS5], F32, kind="ExternalOutput")

    with tile.TileContext(nc) as tc:
        with tc.tile_pool(name="consts", bufs=1) as cp, \
             tc.tile_pool(name="stream", bufs=8) as sp, \
             tc.tile_pool(name="acc", bufs=4) as ap_, \
             tc.tile_pool(name="work", bufs=1) as wp, \
             tc.tile_pool(name="psA", bufs=1, space="PSUM") as psA, \
             tc.tile_pool(name="psB", bufs=2, space="PSUM") as psB, \
             tc.tile_pool(name="psC", bufs=3, space="PSUM") as psC:

            # ---- constants: two packed DMAs + on-device iota ---------------
            with tc.high_priority(offset=2_000_000):
                pk1_sb = cp.tile([128, F1], F32)
                nc.sync.dma_start(out=pk1_sb[:], in_=pk1[:])
                pk2_sb = cp.tile([NS5, F2], F32)
                nc.sync.dma_start(out=pk2_sb[:], in_=pk2[:])
            iota_sb = cp.tile([NS, S], F32)
            nc.gpsimd.iota(iota_sb[:], pattern=[[1, S]], base=0,
                           channel_multiplier=0,
                           allow_small_or_imprecise_dtypes=True)

            wenc_v = pk2_sb[:, 0:512]
            bsel_v = pk2_sb[:NS, O_BSEL:O_BSEL + NS5]
            vwblk_v = pk2_sb[:, O_VWB:O_VWB + NS]
            eblkm_v = pk2_sb[:, O_EBM:O_EBM + NS]
            lenf_v = pk2_sb[:NS, O_LEN:O_LEN + 1]
            nonwin_v = pk2_sb[:1, O_NW:O_NW + NS]
            mfloor_v = pk2_sb[:1, O_MF:O_MF + NS]
            vwin_v = pk2_sb[:1, O_VW:O_VW + NS5]
            negoff_v = pk2_sb[:1, O_NO:O_NO + NS5]
            ident_v = pk1_sb[:, O_ID:O_ID + 128]

            # ---- big stream: vsum[slot] = sum of that slot's valid rows ----
            vsum_ps = psA.tile([NS, H], F32, tag="vsum")
            for g in range(TB):
                t = sp.tile([128, GRP, H], F32, tag="stream")
                nc.sync.dma_start(out=t[:], in_=penc[g])
                q2 = ap_.tile([128, H], F32, tag="q2")
                # high priority: free stream-tile slots promptly so DMA never
                # stalls on pool bufs (DVE otherwise runs the scalar chain first)
                with tc.high_priority(offset=1_000_000):
                    nc.vector.tensor_tensor(out=q2[:], in0=t[:, 0, :], in1=t[:, 1, :], op=OP.add)
                nc.tensor.matmul(vsum_ps[:], pk1_sb[:, O_SEL + g * NS:O_SEL + (g + 1) * NS],
                                 q2[:], start=(g == 0), stop=(g == TB - 1),
                                 skip_group_check=True)

            # ---- q = dh @ Wa  -> window scores ----------------------------
            q_ps = psB.tile([NS, H], F32, tag="bigtmp")
            for k in range(4):
                nc.tensor.matmul(q_ps[:], pk1_sb[:, O_DHT + k * NS:O_DHT + (k + 1) * NS],
                                 pk1_sb[:, k * 512:(k + 1) * 512],
                                 start=(k == 0), stop=(k == 3), skip_group_check=True)
            q_sb = wp.tile([NS, H], F32)
            nc.scalar.copy(out=q_sb[:], in_=q_ps[:])

            qrep_ps = psB.tile([NS5, H], F32, tag="bigtmp")
            nc.tensor.matmul(qrep_ps[:], bsel_v, q_sb[:], start=True, stop=True,
                             skip_group_check=True)
            prod_sb = wp.tile([NS5, H], F32)
            nc.vector.tensor_tensor(out=prod_sb[:], in0=wenc_v, in1=qrep_ps[:], op=OP.mult)
            svec_sb = wp.tile([NS5, 1], F32)
            nc.vector.reduce_sum(out=svec_sb[:], in_=prod_sb[:], axis=mybir.AxisListType.X)

            # scores to free axis: s_row[0, slot*5+j]
            srow_ps = psC.tile([1, NS5], F32, tag="smalltmp")
            nc.tensor.transpose(srow_ps[:], svec_sb[:], ident_v[:NS5, :NS5])
            srow_sb = wp.tile([1, NS5], F32)
            nc.scalar.copy(out=srow_sb[:], in_=srow_ps[:])

            # masked scores: s*vwin + (vwin-1)*1e9
            sm_sb = wp.tile([1, NS5], F32)
            nc.vector.tensor_tensor(out=sm_sb[:], in0=srow_sb[:], in1=vwin_v, op=OP.mult)
            nc.vector.tensor_tensor(out=sm_sb[:], in0=sm_sb[:], in1=negoff_v, op=OP.add)

            # m = max(max_j s_masked, mfloor)
            mrow_sb = wp.tile([1, NS], F32)
            nc.vector.reduce_max(out=mrow_sb[:],
                                 in_=sm_sb[:].rearrange("p (s j) -> p s j", j=WINDOW),
                                 axis=mybir.AxisListType.X)
            nc.vector.tensor_tensor(out=mrow_sb[:], in0=mrow_sb[:], in1=mfloor_v, op=OP.max)

            # broadcast m to 5 window lanes via PE (transpose + bsel matmul)
            mcol_ps = psC.tile([NS, 1], F32, tag="smalltmp")
            nc.tensor.transpose(mcol_ps[:], mrow_sb[:], ident_v[:1, :1])
            mcol_sb = wp.tile([NS, 1], F32)
            nc.scalar.copy(out=mcol_sb[:], in_=mcol_ps[:])
            m5_ps = psC.tile([1, NS5], F32, tag="smalltmp")
            nc.tensor.matmul(m5_ps[:], mcol_sb[:], bsel_v, start=True, stop=True,
                             skip_group_check=True)

            # e = exp(s_masked - m)
            d_sb = wp.tile([1, NS5], F32)
            nc.vector.tensor_tensor(out=d_sb[:], in0=sm_sb[:], in1=m5_ps[:], op=OP.subtract)
            e_sb = wp.tile([1, NS5], F32)
            nc.scalar.activation(out=e_sb[:], in_=d_sb[:], func=AF.Exp)

            # E = sum_j e ; expm = exp(-m) ; Z = nonwin*expm + E ; inv = 1/Z
            E_sb = wp.tile([1, NS], F32)
            nc.vector.reduce_sum(out=E_sb[:],
                                 in_=e_sb[:].rearrange("p (s j) -> p s j", j=WINDOW),
                                 axis=mybir.AxisListType.X)
            expm_sb = wp.tile([1, NS], F32)
            nc.scalar.activation(out=expm_sb[:], in_=mrow_sb[:], func=AF.Exp, scale=-1.0)
            z_sb = wp.tile([1, NS], F32)
            nc.vector.tensor_tensor(out=z_sb[:], in0=nonwin_v, in1=expm_sb[:], op=OP.mult)
            nc.vector.tensor_tensor(out=z_sb[:], in0=z_sb[:], in1=E_sb[:], op=OP.add)
            inv_sb = wp.tile([1, NS], F32)
            nc.vector.reciprocal(out=inv_sb[:], in_=z_sb[:])
            t1_sb = wp.tile([1, NS], F32)  # base weight = exp(-m)/Z
            nc.vector.tensor_tensor(out=t1_sb[:], in0=inv_sb[:], in1=expm_sb[:], op=OP.mult)

            # per-partition scalar columns
            t1c_ps = psC.tile([NS, 1], F32, tag="smalltmp")
            nc.tensor.transpose(t1c_ps[:], t1_sb[:], ident_v[:1, :1])
            t1c_sb = wp.tile([NS, 1], F32)
            nc.scalar.copy(out=t1c_sb[:], in_=t1c_ps[:])
            invc_ps = psC.tile([NS, 1], F32, tag="smalltmp")
            nc.tensor.transpose(invc_ps[:], inv_sb[:], ident_v[:1, :1])
            invc_sb = wp.tile([NS, 1], F32)
            nc.scalar.copy(out=invc_sb[:], in_=invc_ps[:])

            # window weights wwin = e * inv (broadcast inv via PE)
            inv5_ps = psC.tile([1, NS5], F32, tag="smalltmp")
            nc.tensor.matmul(inv5_ps[:], invc_sb[:], bsel_v, start=True, stop=True,
                             skip_group_check=True)
            wwin_sb = wp.tile([1, NS5], F32)
            nc.vector.tensor_tensor(out=wwin_sb[:], in0=e_sb[:], in1=inv5_ps[:], op=OP.mult)
            nc.sync.dma_start(out=owwin[:], in_=wwin_sb[:])

            # eblk[k, slot] = e_k * (k//5 == slot) for the C matmul
            eT_ps = psC.tile([NS5, 1], F32, tag="smalltmp")
            nc.tensor.transpose(eT_ps[:], e_sb[:], ident_v[:1, :1])
            eT_sb = wp.tile([NS5, 1], F32)
            nc.scalar.copy(out=eT_sb[:], in_=eT_ps[:])
            eblk_sb = wp.tile([NS5, NS], F32)
            nc.vector.tensor_scalar(out=eblk_sb[:], in0=eblkm_v,
                                    scalar1=eT_sb[:], scalar2=None, op0=OP.mult)

            # A = sum_{winvalid} enc_j ; C = sum e_j * enc_j
            a_ps = psB.tile([NS, H], F32, tag="AC")
            nc.tensor.matmul(a_ps[:], vwblk_v, wenc_v, start=True, stop=True,
                             skip_group_check=True)
            c_ps = psB.tile([NS, H], F32, tag="AC")
            nc.tensor.matmul(c_ps[:], eblk_sb[:], wenc_v, start=True, stop=True,
                             skip_group_check=True)

            # ctx = (vsum - A)*t1 + C*inv = vsum*t1 + (C*inv - A*t1)
            # h1 = C*inv - A*t1 is ready long before vsum, so the tail after the
            # final stream matmul is a single fused scalar_tensor_tensor.
            d3_sb = wp.tile([NS, H], F32)
            nc.vector.tensor_scalar(out=d3_sb[:], in0=c_ps[:],
                                    scalar1=invc_sb[:], scalar2=None, op0=OP.mult)
            at_sb = wp.tile([NS, H], F32)
            nc.vector.tensor_scalar(out=at_sb[:], in0=a_ps[:],
                                    scalar1=t1c_sb[:], scalar2=None, op0=OP.mult)
            h1_sb = wp.tile([NS, H], F32)
            nc.vector.tensor_tensor(out=h1_sb[:], in0=d3_sb[:], in1=at_sb[:], op=OP.subtract)
            ctx_sb = wp.tile([NS, H], F32)
            nc.vector.scalar_tensor_tensor(out=ctx_sb[:], in0=vsum_ps[:],
                                           scalar=t1c_sb[:], in1=h1_sb[:],
                                           op0=OP.mult, op1=OP.add)
            nc.sync.dma_start(out=octx[:], in_=ctx_sb[:])

            # attn base row: (iota < len) * base ; window region fixed on host
            attn_sb = wp.tile([NS, S], F32)
            nc.vector.tensor_scalar(out=attn_sb[:], in0=iota_sb[:],
                                    scalar1=lenf_v, scalar2=t1c_sb[:],
                                    op0=OP.is_lt, op1=OP.mult)
            nc.sync.dma_start(out=oattn[:], in_=attn_sb[:])

    nc.compile()
    return nc


def kernel(**inputs):
    dh = np.ascontiguousarray(np.asarray(inputs["decoder_hidden"], dtype=np.float32))
    enc = np.ascontiguousarray(np.asarray(inputs["encoder_outputs"], dtype=np.float32))
    Wp = np.asarray(inputs["Wp"], dtype=np.float32)
    bp = np.asarray(inputs["bp"], dtype=np.float32)
    Wa = np.ascontiguousarray(np.asarray(inputs["Wa"], dtype=np.float32))
    lens = np.asarray(inputs["input_lengths"]).astype(np.int64)

    centers = _centers(dh, Wp, bp).astype(np.int64)
    left = np.maximum(0, centers - HALF)
    right = np.minimum(S, centers + HALF + 1)
    width = right - left

    slots, NS = _assign_batches(lens)
    NS5 = NS * WINDOW

    # rows per core with each slot padded to a multiple of GRP rows
    rows_c = [sum(int(-(-lens[b] // GRP) * GRP) for b in sl) for sl in slots]
    TB = max(1, -(-max(rows_c) // TILE_ROWS))
    R = TB * TILE_ROWS

    F1 = 2048 + 128 + TB * NS + 4 * NS
    O_ID, O_SEL, O_DHT = 2048, 2048 + 128, 2048 + 128 + TB * NS
    F2 = 512 + NS5 + NS + NS + 1 + NS + NS + NS5 + NS5
    O_BSEL = 512
    O_VWB = O_BSEL + NS5
    O_EBM = O_VWB + NS
    O_LEN = O_EBM + NS
    O_NW = O_LEN + 1
    O_MF = O_NW + NS
    O_VW = O_MF + NS
    O_NO = O_VW + NS5

    ident_h = np.eye(128, dtype=np.float32)
    bsel_h = np.zeros((NS, NS5), np.float32)
    eblkm_h = np.zeros((NS5, NS), np.float32)
    for s_ in range(NS):
        bsel_h[s_, s_ * WINDOW:(s_ + 1) * WINDOW] = 1.0
        eblkm_h[s_ * WINDOW:(s_ + 1) * WINDOW, s_] = 1.0

    in_maps = []
    for c in range(N_CORES):
        sl = slots[c]
        packed = np.zeros((R, H), np.float32)
        rowslot = np.full(R, -1, np.int64)
        wenc_h = np.zeros((NS5, H), np.float32)
        dhT_h = np.zeros((H, NS), np.float32)
        lenf_h = np.zeros((NS,), np.float32)
        nonwin_h = np.ones((NS,), np.float32)
        mfloor_h = np.zeros((NS,), np.float32)
        vwin_h = np.zeros((NS5,), np.float32)
        vwblk_h = np.zeros((NS5, NS), np.float32)
        off = 0
        for s_, b in enumerate(sl):
            L = int(lens[b])
            packed[off:off + L] = enc[b, :L]
            rowslot[off:off + L] = s_
            off += -(-L // GRP) * GRP
            dhT_h[:, s_] = dh[b]
            lenf_h[s_] = np.float32(L)
            nwv = 0
            for j in range(WINDOW):
                p = int(left[b]) + j
                if j < int(width[b]) and p < L:
                    wenc_h[s_ * WINDOW + j] = enc[b, p]
                    vwin_h[s_ * WINDOW + j] = 1.0
                    vwblk_h[s_ * WINDOW + j, s_] = 1.0
                    nwv += 1
            nonwin_h[s_] = np.float32(L - nwv)
            mfloor_h[s_] = np.float32(0.0) if (L - nwv) > 0 else np.float32(-1e30)
        negoff_h = (vwin_h - 1.0) * np.float32(1e9)

        # interleaved layout: row (g*256 + p*2 + k) -> penc[g, p, k, :]
        penc_h = packed.reshape(TB, 128, GRP, H)
        gs = rowslot.reshape(TB, 128, GRP)[:, :, 0]  # slot of each row-pair
        selg_h = np.zeros((TB, 128, NS), np.float32)
        gi, pi = np.nonzero(gs >= 0)
        selg_h[gi, pi, gs[gi, pi]] = 1.0

        pk1_h = np.zeros((128, F1), np.float32)
        pk1_h[:, 0:2048] = Wa.reshape(4, 128, H).transpose(1, 0, 2).reshape(128, 2048)
        pk1_h[:, O_ID:O_ID + 128] = ident_h
        pk1_h[:, O_SEL:O_SEL + TB * NS] = selg_h.transpose(1, 0, 2).reshape(128, TB * NS)
        pk1_h[:, O_DHT:O_DHT + 4 * NS] = dhT_h.reshape(4, 128, NS).transpose(1, 0, 2).reshape(128, 4 * NS)

        pk2_h = np.zeros((NS5, F2), np.float32)
        pk2_h[:, 0:512] = wenc_h
        pk2_h[:NS, O_BSEL:O_BSEL + NS5] = bsel_h
        pk2_h[:, O_VWB:O_VWB + NS] = vwblk_h
        pk2_h[:, O_EBM:O_EBM + NS] = eblkm_h
        pk2_h[:NS, O_LEN] = lenf_h
        pk2_h[0, O_NW:O_NW + NS] = nonwin_h
        pk2_h[0, O_MF:O_MF + NS] = mfloor_h
        pk2_h[0, O_VW:O_VW + NS5] = vwin_h
        pk2_h[0, O_NO:O_NO + NS5] = negoff_h

        in_maps.append({"penc": penc_h, "pk1": pk1_h, "pk2": pk2_h})

    nc = _build_program(NS, TB)
    global last_nc, last_run
    last_nc = nc
    _r = run_bass_kernel_spmd(nc, in_maps, list(range(N_CORES)))
    last_run = _r
    res = _r.results

    ctx_full = np.zeros((B, H), np.float32)
    attn_full = np.zeros((B, S), np.float32)
    for c in range(N_CORES):
        for s_, b in enumerate(slots[c]):
            ctx_full[b] = res[c]["octx"][s_]
            attn_full[b] = res[c]["oattn"][s_]
            w = int(width[b])
            lo = int(left[b])
            attn_full[b, lo:lo + w] = res[c]["owwin"][0, s_ * WINDOW:s_ * WINDOW + w]
    return ctx_full, attn_full
